# revision 1
# baseline (speedup 1.0000x reference)
"""Trainium2 Bass kernel for nn_Attention_module_52166672777937.

Data-parallel over batch across 8 NeuronCores (4 sequences per core).

Algorithmic restructuring (numerically validated against the reference):
the module only consumes the attention output at the LAST valid position
of each sequence (take_along_axis with lengths-1), and attention is
causal, so only ONE query row per sequence matters.  Consequences:

  * q is computed for a single position per sequence.
  * K is never materialized: scores = (qblk.T @ Wk) @ x.T, using
    associativity of the K projection with the score contraction.
  * softmax runs over [H=8, L] scores per sequence (no L x L matrix).
  * ctx = softmax(scores) @ V needs V = x @ Wv.T for all positions -- the
    dominant matmul, kept on TensorE at fp32r full rate.

Device layout: x is built in transposed [E, L] layout directly via a
one-hot matmul gather (onehot[c, l] = (data[l] == c), x.T = emb.T @
onehot + pe.T), which feeds both the score matmul and the V projection
without any transposes of large tensors.
"""

import math
import sys

import ml_dtypes
import numpy as np

sys.path.insert(0, "/opt/trn_rl_repo")

import concourse.bacc as bacc
import concourse.bass as bass
import concourse.mybir as mybir
import concourse.tile as tile
from concourse.bass_utils import run_bass_kernel_spmd

dt = mybir.dt
AF = mybir.ActivationFunctionType
ALU = mybir.AluOpType
PSUM = bass.MemorySpace.PSUM

N_CORES = 8
B, L = 32, 1000
LP = 1024                 # padded sequence length (2 x 512 column tiles)
TW = 512                  # column-tile width (max fp32 moving operand / PSUM bank)
NT = LP // TW             # column tiles per sequence
BPC = B // N_CORES        # sequences per core
NCH = 256                 # vocabulary
E = 512                   # embedding dim
D = 512                   # d_model
NH, DH = 8, 64            # heads
HS = 512                  # pred hidden size
NOUT = 8
NEG = -1.0e30
SCALE = 1.0 / math.sqrt(DH)


def _build():
    nc = bacc.Bacc(
        "TRN2", target_bir_lowering=False, debug=False, num_devices=N_CORES
    )

    f32 = dt.float32
    f32r = dt.float32r
    bf16 = dt.bfloat16
    # --- packed inputs (few wide DMAs instead of many narrow ones) ------
    # bf16s: data+idxlast row; emb; wqT|wk|wvT; peT; id8b
    d_drow = nc.dram_tensor("drow", [1, BPC * LP + BPC], bf16,
                            kind="ExternalInput")
    d_emb = nc.dram_tensor("emb", [NCH, E], bf16, kind="ExternalInput")
    d_wb = nc.dram_tensor("wb", [E, 3 * D], bf16, kind="ExternalInput")
    d_peT = nc.dram_tensor("peT", [E, LP], bf16, kind="ExternalInput")
    d_id8b = nc.dram_tensor("id8b", [NH, NH], bf16, kind="ExternalInput")
    # f32r: w1T|w2T ; ones8
    d_wr = nc.dram_tensor("wr", [D, HS + NOUT], f32r, kind="ExternalInput")
    d_ones8 = nc.dram_tensor("ones8", [NH, 1], f32r, kind="ExternalInput")
    # f32: pelastT|hmask (512-row); bq|b1|cvals (128-row); plast|b2|iota (8-row)
    d_fA = nc.dram_tensor("fA", [D, BPC + NH], f32, kind="ExternalInput")
    d_fB = nc.dram_tensor("fB", [128, 42], f32, kind="ExternalInput")
    d_fC = nc.dram_tensor("fC", [NH, BPC + 1 + LP], f32, kind="ExternalInput")
    d_out = nc.dram_tensor("out", [1, BPC], f32, kind="ExternalOutput")

    with tile.TileContext(nc) as tc:
        with (
            tc.tile_pool(name="const", bufs=1) as cp,
            tc.tile_pool(name="work", bufs=2) as wp,
            tc.tile_pool(name="psx", bufs=2, space=PSUM) as psx,
            tc.tile_pool(name="psv", bufs=2, space=PSUM) as psv,
            tc.tile_pool(name="pss", bufs=2, space=PSUM) as pss,
            tc.tile_pool(name="psc", bufs=1, space=PSUM) as psc,
            tc.tile_pool(name="psd", bufs=1, space=PSUM) as psd,
        ):
            # ---------------- constant loads (packed) -------------------
            # issued in order of first use so compute starts early:
            # fB(cvals) -> db0 -> emb -> wq -> wk -> peT -> wv -> ...
            fB_sb = cp.tile([128, 42], f32, name="fB", tag="fB")
            nc.sync.dma_start(out=fB_sb[:], in_=d_fB[:])
            bq_sb = fB_sb[:, 0:4]
            b1_sb = fB_sb[:, 4:8]
            cvals_sb = fB_sb[:, 8:10]
            maskT_sb = fB_sb[:, 10:42]
            ones128_sb = cp.tile([128, 1], bf16, name="ones128",
                                 tag="ones128")
            nc.vector.memset(ones128_sb[:], 1.0)
            dbb = []
            for b in range(BPC):
                t = cp.tile([128, LP], bf16, name=f"db{b}", tag=f"db{b}")
                dbb.append(t)
            nc.sync.dma_start(
                out=dbb[0][:],
                in_=d_drow[:, 0:LP].to_broadcast((128, LP)),
            )
            embp_sb = cp.tile([128, 2, E], bf16, name="embp", tag="embp")
            nc.sync.dma_start(
                out=embp_sb[:],
                in_=d_emb[:].rearrange("(c p) n -> p c n", p=128),
            )
            emb_sb = [embp_sb[:, c, :] for c in range(2)]
            wqp_sb = cp.tile([128, 4, D], bf16, name="wqp", tag="wqp")
            nc.sync.dma_start(
                out=wqp_sb[:],
                in_=d_wb[:, 0:D].rearrange("(c p) n -> p c n", p=128),
            )
            wqT_sb = [wqp_sb[:, e, :] for e in range(4)]
            wkp_sb = cp.tile([128, 4, D], bf16, name="wkp", tag="wkp")
            nc.sync.dma_start(
                out=wkp_sb[:],
                in_=d_wb[:, D:2 * D].rearrange("(c p) n -> p c n", p=128),
            )
            wk_sb = [wkp_sb[:, c, :] for c in range(4)]
            peTp_sb = cp.tile([128, 4, LP], bf16, name="peTp", tag="peTp")
            nc.sync.dma_start(
                out=peTp_sb[:],
                in_=d_peT[:].rearrange("(c p) n -> p c n", p=128),
            )
            peT_sb = [peTp_sb[:, e, :] for e in range(4)]
            wvp_sb = cp.tile([128, 4, D], bf16, name="wvp", tag="wvp")
            nc.sync.dma_start(
                out=wvp_sb[:],
                in_=d_wb[:, 2 * D:3 * D].rearrange("(c p) n -> p c n", p=128),
            )
            wvT_sb = [wvp_sb[:, e, :] for e in range(4)]
            idxb_sb = cp.tile([128, BPC], bf16, name="idxb", tag="idxb")
            nc.sync.dma_start(
                out=idxb_sb[:],
                in_=d_drow[:, BPC * LP:].to_broadcast((128, BPC)),
            )
            fA_sb = cp.tile([128, 4, BPC + NH], f32, name="fA", tag="fA")
            nc.sync.dma_start(
                out=fA_sb[:], in_=d_fA[:].rearrange("(c p) n -> p c n", p=128)
            )
            pelT_sb = [fA_sb[:, m, 0:BPC] for m in range(4)]
            hmask_sb = [fA_sb[:, m, BPC:BPC + NH] for m in range(4)]
            fC_sb = cp.tile([NH, BPC + 1 + LP], f32, name="fC", tag="fC")
            nc.sync.dma_start(out=fC_sb[:], in_=d_fC[:])
            plast_sb = fC_sb[:, 0:BPC]
            b2_sb = fC_sb[:, BPC:BPC + 1]
            iota_sb = fC_sb[:, BPC + 1:]
            id8b_sb = cp.tile([NH, NH], bf16, name="id8b", tag="id8b")
            nc.sync.dma_start(out=id8b_sb[:], in_=d_id8b[:])
            for b in range(1, BPC):
                nc.sync.dma_start(
                    out=dbb[b][:],
                    in_=d_drow[:, b * LP:(b + 1) * LP].to_broadcast(
                        (128, LP)),
                )
            drow_sb = cp.tile([1, BPC * LP + BPC], bf16, name="drow",
                              tag="drow")
            nc.sync.dma_start(out=drow_sb[:], in_=d_drow[:])
            wr_sb = cp.tile([128, 4, HS + NOUT], f32r, name="wr", tag="wr")
            nc.sync.dma_start(
                out=wr_sb[:], in_=d_wr[:].rearrange("(c p) n -> p c n", p=128)
            )
            w1T_sb = [wr_sb[:, m, 0:HS] for m in range(4)]
            w2T_sb = [wr_sb[:, m, HS:HS + NOUT] for m in range(4)]
            ones8_sb = cp.tile([NH, 1], f32r, name="ones8", tag="ones8")
            nc.sync.dma_start(out=ones8_sb[:], in_=d_ones8[:])

            madd_sb = [None] * BPC

            def emit_gather(b, t):
                # one-hot + x.T tile for (sequence b, column tile t)
                oh = []
                for c in range(2):
                    o = wp.tile([128, TW], bf16, name=f"oh{b}_{t}_{c}",
                                tag="oh", bufs=6)
                    nc.vector.tensor_scalar(
                        o[:], dbb[b][:, t * TW:(t + 1) * TW],
                        cvals_sb[:, c:c + 1], None, ALU.is_equal,
                    )
                    oh.append(o)
                xT = []
                for e in range(4):
                    p = psx.tile([128, TW], f32, name=f"xtp{b}_{t}_{e}",
                                 tag="xtp")
                    for c in range(2):
                        nc.tensor.matmul(
                            p[:], (emb_sb[c][:, e * 128:(e + 1) * 128]),
                            (oh[c][:]), start=(c == 0), stop=(c == 1),
                        )
                    x = wp.tile([128, TW], bf16, name=f"xT{b}_{t}_{e}",
                                tag=f"xT{e}", bufs=3)
                    nc.vector.tensor_tensor(
                        x[:], p[:], peT_sb[e][:, t * TW:(t + 1) * TW],
                        ALU.add,
                    )
                    xT.append(x)
                return xT

            # first tile's gather depends only on the earliest DMAs; emit it
            # ahead of the serial q-prep chain so the PE queue head has work
            xT_first = emit_gather(0, 0)

            # ---------------- x_last gather -> q ------------------------
            ohl = []
            for c in range(2):
                t = cp.tile([128, BPC], bf16, name=f"ohl{c}", tag=f"ohl{c}")
                nc.vector.tensor_scalar(
                    t[:], idxb_sb[:], cvals_sb[:, c:c + 1], None, ALU.is_equal
                )
                ohl.append(t)
            # x_last.T [E, BPC] = emb.T @ onehot_last + pe_last.T
            xlast_sb = []
            for e in range(4):
                p = psx.tile([128, BPC], f32, name=f"xlp{e}", tag="xtp")
                for c in range(2):
                    nc.tensor.matmul(
                        p[:], (emb_sb[c][:, e * 128:(e + 1) * 128]),
                        (ohl[c][:]), start=(c == 0), stop=(c == 1),
                    )
                t = cp.tile([128, BPC], bf16, name=f"xlast{e}", tag=f"xlast{e}")
                nc.vector.tensor_tensor(t[:], p[:], pelT_sb[e][:], ALU.add)
                xlast_sb.append(t)
            # q.T [D, BPC] = Wq @ x_last.T + bq
            qT_sb = []
            for d in range(4):
                p = psv.tile([128, BPC], f32, name=f"qp{d}", tag="vp")
                for e in range(4):
                    nc.tensor.matmul(
                        p[:], (wqT_sb[e][:, d * 128:(d + 1) * 128]),
                        (xlast_sb[e][:]), start=(e == 0), stop=(e == 3),
                    )
                t = cp.tile([128, BPC], f32, name=f"qT{d}", tag=f"qT{d}")
                nc.vector.tensor_scalar(t[:], p[:], bq_sb[:, d:d + 1], None,
                                        ALU.add)
                qT_sb.append(t)

            # ---------------- main loop over sequences ------------------
            out_sb = cp.tile([1, BPC], f32, name="out_sb", tag="out_sb")
            ctxT_sb = [cp.tile([128, BPC], f32r, name=f"ctxT{m}", tag=f"ctxT{m}")
                       for m in range(4)]
            for b in range(BPC):
                # --- per-sequence qkvec = qblk.T @ Wk (K never formed) --
                qblk = []
                for d in range(4):
                    t = cp.tile([128, NH], bf16, name=f"qblk{b}_{d}",
                                tag=f"qblk{b}_{d}")
                    nc.vector.tensor_scalar(
                        t[:], hmask_sb[d][:], qT_sb[d][:, b:b + 1], None,
                        ALU.mult,
                    )
                    qblk.append(t)
                qkvp = pss.tile([NH, E], f32, name=f"qkvp{b}", tag="sp")
                for d in range(4):
                    nc.tensor.matmul(
                        qkvp[:], (qblk[d][:]), (wk_sb[d][:]),
                        start=(d == 0), stop=(d == 3),
                    )
                qkv_sb = wp.tile([NH, E], bf16, name=f"qkv{b}", tag="qkv",
                                 bufs=2)
                nc.vector.tensor_copy(qkv_sb[:], qkvp[:])
                qkvT = []
                for e in range(4):
                    tp = pss.tile([128, NH], bf16, name=f"qkvTp{b}_{e}",
                                  tag="sp")
                    nc.tensor.transpose(
                        tp[:], qkv_sb[:, e * 128:(e + 1) * 128], id8b_sb[:]
                    )
                    t = cp.tile([128, NH], bf16, name=f"qkvT{b}_{e}",
                                tag=f"qkvT{b}_{e}")
                    nc.vector.tensor_copy(t[:], tp[:])
                    qkvT.append(t)

                # --- attention over the sequence ------------------------
                ctxp = psc.tile([NH, D], f32, name=f"ctx{b}", tag="cp")
                dnp = psd.tile([1, NH], f32, name=f"dn{b}", tag="dn")
                for t in range(NT):
                    xT = xT_first if (b == 0 and t == 0) else emit_gather(b, t)
                    # per l-chunk: transposed scores [l, h], exp with the
                    # causal mask as the per-partition ACT bias, V, ctx
                    for lc in range(4):
                        slp = pss.tile([128, NH], f32,
                                       name=f"sl{b}_{t}_{lc}", tag="sp")
                        for e in range(4):
                            nc.tensor.matmul(
                                slp[:],
                                (xT[e][:, lc * 128:(lc + 1) * 128]),
                                (qkvT[e][:]),
                                start=(e == 0), stop=(e == 3),
                            )
                        aT = wp.tile([128, NH], bf16, name=f"aT{b}_{t}_{lc}",
                                     tag="aT", bufs=8)
                        mcol = b * 8 + t * 4 + lc
                        nc.scalar.activation(
                            aT[:], slp[:], AF.Exp, scale=SCALE,
                            bias=maskT_sb[:, mcol:mcol + 1],
                        )
                        nc.tensor.matmul(
                            dnp[:], ones128_sb[:], aT[:],
                            start=(t == 0 and lc == 0),
                            stop=(t == NT - 1 and lc == 3),
                        )
                        vp = psv.tile([128, D], f32, name=f"vp{b}_{t}_{lc}",
                                      tag="vp")
                        for e in range(4):
                            nc.tensor.matmul(
                                vp[:],
                                (xT[e][:, lc * 128:(lc + 1) * 128]),
                                (wvT_sb[e][:]),
                                start=(e == 0), stop=(e == 3),
                            )
                        v = wp.tile([128, D], bf16, name=f"v{b}_{t}_{lc}",
                                    tag="v", bufs=4)
                        if lc % 2 == 0:
                            nc.vector.tensor_copy(v[:], vp[:])
                        else:
                            nc.scalar.copy(v[:], vp[:])
                        nc.tensor.matmul(
                            ctxp[:], (aT[:]), (v[:]),
                            start=(t == 0 and lc == 0),
                            stop=(t == NT - 1 and lc == 3),
                        )
                # normalize ctx rows by the masked softmax denominator;
                # den is [1, 8] (summed over partitions via ones-matmul),
                # transpose to [8, 1] with a K=1 matmul (identity scalar
                # borrowed from the iota column whose value is 1.0)
                dn_sb = wp.tile([1, NH], f32, name=f"dns{b}", tag="dns",
                                bufs=2)
                nc.vector.tensor_copy(dn_sb[:], dnp[:])
                dTp = pss.tile([NH, 1], f32, name=f"dTp{b}", tag="sp")
                nc.tensor.transpose(
                    dTp[:], dn_sb[:], fC_sb[0:1, BPC + 2:BPC + 3]
                )
                dsum = wp.tile([NH, 1], f32, name=f"dsum{b}", tag="dsum",
                               bufs=2)
                nc.vector.tensor_copy(dsum[:], dTp[:])
                rec = wp.tile([NH, 1], f32, name=f"rec{b}", tag="rec", bufs=2)
                nc.vector.reciprocal(rec[:], dsum[:])
                ctx_sb = wp.tile([NH, D], bf16, name=f"ctxs{b}", tag="ctxs",
                                 bufs=2)
                nc.scalar.activation(ctx_sb[:], ctxp[:], AF.Copy,
                                     scale=rec[:])
                # extract block-diagonal -> ctx.T [D, BPC] column b
                for m in range(4):
                    tp = pss.tile([128, NH], bf16, name=f"ctp{b}_{m}", tag="sp")
                    nc.tensor.transpose(
                        tp[:], ctx_sb[:, m * 128:(m + 1) * 128], id8b_sb[:]
                    )
                    scr = wp.tile([128, NH], f32, name=f"scr{b}_{m}",
                                  tag="scr", bufs=2)
                    nc.vector.tensor_tensor(scr[:], tp[:], hmask_sb[m][:],
                                            ALU.mult)
                    with nc.allow_low_precision("fp32 accum, fp32r round"):
                        nc.vector.tensor_reduce(
                            ctxT_sb[m][:, b:b + 1], scr[:],
                            mybir.AxisListType.X, ALU.add,
                        )

            # ---------------- prediction head ---------------------------
            hT_sb = []
            for hc in range(4):
                p = psv.tile([128, BPC], f32, name=f"hp{hc}", tag="vp")
                for m in range(4):
                    nc.tensor.matmul(
                        p[:], (w1T_sb[m][:, hc * 128:(hc + 1) * 128]),
                        (ctxT_sb[m][:]), start=(m == 0), stop=(m == 3),
                    )
                t1 = wp.tile([128, BPC], f32, name=f"t1_{hc}", tag="t1",
                             bufs=2)
                nc.vector.tensor_scalar(t1[:], p[:], b1_sb[:, hc:hc + 1],
                                        None, ALU.add)
                ht = cp.tile([128, BPC], f32r, name=f"hT{hc}", tag=f"hT{hc}")
                nc.vector.scalar_tensor_tensor(
                    ht[:], t1[:], 0.01, t1[:], ALU.mult, ALU.max
                )
                hT_sb.append(ht)
            r2p = pss.tile([NOUT, BPC], f32, name="r2p", tag="sp")
            for hc in range(4):
                nc.tensor.matmul(
                    r2p[:], (w2T_sb[hc][:]), (hT_sb[hc][:]),
                    start=(hc == 0), stop=(hc == 3),
                )
            r_sb = cp.tile([NOUT, BPC], f32r, name="r_sb", tag="r_sb")
            nc.vector.tensor_scalar(r_sb[:], r2p[:], b2_sb[:], 0.0,
                                    ALU.add, ALU.max)
            mp = pss.tile([1, BPC], f32, name="mp", tag="sp")
            nc.tensor.matmul(mp[:], (ones8_sb[:]), (r_sb[:]))
            mt = cp.tile([1, BPC], f32, name="mt", tag="mt")
            nc.vector.tensor_scalar(mt[:], mp[:], 1.0 / NOUT, None, ALU.mult)
            nc.vector.scalar_tensor_tensor(
                out_sb[:], mt[:], 0.01, mt[:], ALU.mult, ALU.max
            )
            nc.sync.dma_start(out=d_out[:], in_=out_sb[:])

    nc.compile()
    return nc


_CACHE = {}


def _get_module():
    if "nc" not in _CACHE:
        _CACHE["nc"] = _build()
    return _CACHE["nc"]


def _pos_encoding():
    pos = np.arange(L, dtype=np.float32)[:, None]
    div = np.exp(
        np.arange(0, D, 2, dtype=np.float32) * (-math.log(10000.0) / D)
    )
    pe = np.zeros((L, D), np.float32)
    pe[:, 0::2] = np.sin(pos * div)
    pe[:, 1::2] = np.cos(pos * div)
    return pe


def make_in_maps(data, lengths, emb, Wq, bq, Wk, bk, Wv, bv, W1, b1, W2, b2):
    # the kernel folds the K-projection into the score contraction; a
    # nonzero bk would add a per-head constant q.bk_h to the scores, which
    # this build omits (bk is zero for this module).
    assert float(np.abs(np.asarray(bk)).max()) == 0.0
    # V eviction is a plain copy; nonzero bv would need a bias add there.
    assert float(np.abs(np.asarray(bv)).max()) == 0.0

    pe = _pos_encoding()                       # [L, D]
    peT = np.zeros((E, LP), np.float32)
    peT[:, :L] = pe.T

    dpad = np.zeros((B, LP), np.int64)
    dpad[:, :L] = data
    data_f32 = dpad.astype(np.float32)

    p = (np.asarray(lengths).astype(np.int64) - 1)          # [B]
    idxl = np.asarray(data)[np.arange(B), p].astype(np.float32)
    pelT = pe[p].astype(np.float32).T                       # [D, B]

    bfl = ml_dtypes.bfloat16
    wb = np.concatenate(
        [np.asarray(Wq).T, np.asarray(Wk), np.asarray(Wv).T], axis=1
    ).astype(bfl)                                            # [512, 1536]
    wr = np.concatenate(
        [np.asarray(W1).T, np.asarray(W2).T], axis=1
    ).astype(np.float32)                                     # [512, 520]
    fB_head = np.concatenate(
        [np.asarray(bq).reshape(4, 128).T,
         np.asarray(b1).reshape(4, 128).T,
         np.arange(256, dtype=np.float32).reshape(2, 128).T], axis=1
    ).astype(np.float32)                                     # [128, 10]
    iota8 = np.broadcast_to(np.arange(LP, dtype=np.float32), (NH, LP))
    shared = {
        "emb": np.ascontiguousarray(emb, dtype=bfl),
        "wb": np.ascontiguousarray(wb),
        "wr": np.ascontiguousarray(wr),
        "peT": peT.astype(bfl),
        "id8b": np.eye(NH, dtype=bfl),
        "ones8": np.ones((NH, 1), np.float32),
    }
    in_maps = []
    for c in range(N_CORES):
        sl = slice(c * BPC, (c + 1) * BPC)
        m = dict(shared)
        l_of = (np.arange(8)[None, :] * 128
                + np.arange(128)[:, None])                   # [128, 8]
        mT = np.where(
            l_of[:, None, :] > p[sl][None, :, None], -1.0e30, 0.0
        ).reshape(128, BPC * 8).astype(np.float32)
        m["fB"] = np.ascontiguousarray(
            np.concatenate([fB_head, mT], axis=1))           # [128, 42]
        m["drow"] = np.concatenate(
            [data_f32[sl].reshape(-1), idxl[sl]]
        ).reshape(1, -1).astype(bfl)
        m["fA"] = np.ascontiguousarray(np.concatenate(
            [pelT[:, sl], np.repeat(np.eye(NH, dtype=np.float32), DH, axis=0)],
            axis=1)).astype(np.float32)                      # [512, 12]
        m["fC"] = np.ascontiguousarray(np.concatenate(
            [np.broadcast_to(p[sl].astype(np.float32), (NH, BPC)),
             np.asarray(b2).reshape(NOUT, 1).astype(np.float32),
             iota8], axis=1)).astype(np.float32)             # [8, 1029]
        in_maps.append(m)
    return in_maps


def kernel(data, lengths, emb, Wq, bq, Wk, bk, Wv, bv, W1, b1, W2, b2):
    nc = _get_module()
    in_maps = make_in_maps(
        np.asarray(data), np.asarray(lengths), emb, Wq, bq, Wk, bk, Wv, bv,
        W1, b1, W2, b2,
    )
    res = run_bass_kernel_spmd(nc, in_maps, list(range(N_CORES)))
    out = np.concatenate(
        [res.results[c]["out"].reshape(BPC) for c in range(N_CORES)]
    )
    return out.astype(np.float32)



# revision 12
# speedup vs baseline: 1.4019x; 1.4019x over previous
"""Trainium2 Bass kernel for nn_Attention_module_52166672777937.

Length-aware chunk-packed attention, data-parallel over batch on 8 cores.

Only the attention output at the LAST valid position of each sequence is
consumed (take_along_axis with lengths-1) and attention is causal, so per
sequence only ONE query row and the key/value positions 0..len-1 matter.
The baseline exploited the single query; this version additionally skips
all work past each sequence's length:

  * positions are processed in 128-wide CHUNKS; sequence b needs only
    ceil(len_b/128) chunks instead of LP/128 = 8.
  * the 32 sequences are LPT-assigned to the 8 cores (4 seqs each) to
    equalize total chunk counts; every core runs the same static program
    of C = max(core totals) chunks (padded with inert chunks).
  * all owner-dependence (which sequence a chunk belongs to, its causal
    boundary, its positional-encoding rows) lives in per-core packed DMA
    data: the packed char stream, packed pe slices, packed score masks.
    The SPMD program itself is core-uniform.
  * scores for all 4 sequences' 32 head-columns are computed per chunk;
    the packed mask (-1e30 on non-owner columns and beyond-boundary rows)
    zeroes foreign contributions after exp, so softmax denominators and
    ctx accumulate over the whole chunk stream into single [.,32]/[32,.]
    PSUM groups.
  * q-prep and the prediction head are batched over the 4 sequences
    (large-moving matmuls instead of per-sequence slivers).

The kernel is JIT-specialized to the actual lengths at first call (C is
derived from the inputs, the module is cached by C).
"""

import math

import ml_dtypes
import numpy as np
import sys

sys.path.insert(0, "/opt/trn_rl_repo")

import concourse.bacc as bacc
import concourse.bass as bass
import concourse.mybir as mybir
import concourse.tile as tile
from concourse.bass_utils import run_bass_kernel_spmd

dt = mybir.dt
AF = mybir.ActivationFunctionType
ALU = mybir.AluOpType
PSUM = bass.MemorySpace.PSUM

N_CORES = 8
B, L = 32, 1000
CH = 128                  # chunk width (positions)
NCH = 256                 # vocabulary
E = 512                   # embedding dim
D = 512                   # d_model
NH, DH = 8, 64            # heads
HS = 512                  # pred hidden size
NOUT = 8
BPC = B // N_CORES        # sequences per core
NSC = BPC * NH            # score columns (4 seqs x 8 heads)
NEG = -1.0e30
SCALE = 1.0 / math.sqrt(DH)
SENT = 300.0              # padding sentinel char (bf16-exact, not in vocab)

# fmix column layout: cvals 2 | hmask 4*8 | pelT 4*4 | hm32 4*32 | id4 4
FM_CV = 0
FM_HM = 2
FM_PL = FM_HM + 4 * NH
FM_H32 = FM_PL + 4 * BPC
FM_ID4 = FM_H32 + 4 * NSC
FM_W = FM_ID4 + 4
# frow column layout: b1 512 | b2 8 | 1.0
FR_W = HS + NOUT + 1


def _group_widths(C):
    gw = [4] * (C // 4)
    if C % 4:
        gw.append(C % 4)
    return gw


def _build(C):
    gw = _group_widths(C)
    G = len(gw)
    goff = [0]
    for w in gw:
        goff.append(goff[-1] + w)

    nc = bacc.Bacc(
        "TRN2", target_bir_lowering=False, debug=False, num_devices=N_CORES
    )

    f32 = dt.float32
    f32r = dt.float32r
    bf16 = dt.bfloat16

    d_drow = nc.dram_tensor("drow", [1, C * CH + BPC], bf16,
                            kind="ExternalInput")
    d_emb = nc.dram_tensor("emb", [NCH, E], bf16, kind="ExternalInput")
    d_wb = nc.dram_tensor("wb", [E, 3 * D], bf16, kind="ExternalInput")
    d_wr = nc.dram_tensor("wr", [D, HS], f32r, kind="ExternalInput")
    d_w2 = nc.dram_tensor("w2", [HS, NOUT], bf16, kind="ExternalInput")
    d_pe = nc.dram_tensor("pe", [128, C * 4 * CH], bf16, kind="ExternalInput")
    d_mask = nc.dram_tensor("mask", [128, C * NSC], bf16,
                            kind="ExternalInput")
    d_fmix = nc.dram_tensor("fmix", [128, FM_W], f32, kind="ExternalInput")
    d_frow = nc.dram_tensor("frow", [1, FR_W], f32, kind="ExternalInput")
    d_id32 = nc.dram_tensor("id32", [32, 32], bf16, kind="ExternalInput")
    d_out = nc.dram_tensor("out", [BPC, 1], f32, kind="ExternalOutput")

    with tile.TileContext(nc) as tc:
        with (
            tc.tile_pool(name="const", bufs=1) as cp,
            tc.tile_pool(name="work", bufs=2) as wp,
            tc.tile_pool(name="psx", bufs=2, space=PSUM) as psx,
            tc.tile_pool(name="psv", bufs=2, space=PSUM) as psv,
            tc.tile_pool(name="pss", bufs=2, space=PSUM) as pss,
            tc.tile_pool(name="psc", bufs=1, space=PSUM) as psc,
            tc.tile_pool(name="psd", bufs=1, space=PSUM) as psd,
        ):
            # ---------------- constant DMAs (ordered by first use) -------
            fmix_sb = cp.tile([128, FM_W], f32, name="fmix", tag="fmix")
            nc.sync.dma_start(out=fmix_sb[:], in_=d_fmix[:])
            cvals = fmix_sb[:, FM_CV:FM_CV + 2]
            hmask = [fmix_sb[:, FM_HM + 8 * m:FM_HM + 8 * (m + 1)]
                     for m in range(4)]
            pelT = [fmix_sb[:, FM_PL + 4 * m:FM_PL + 4 * (m + 1)]
                    for m in range(4)]
            hm32 = [fmix_sb[:, FM_H32 + NSC * m:FM_H32 + NSC * (m + 1)]
                    for m in range(4)]
            id4f = fmix_sb[0:4, FM_ID4:FM_ID4 + 4]

            idxl_sb = cp.tile([128, BPC], bf16, name="idxl", tag="idxl")
            nc.sync.dma_start(
                out=idxl_sb[:],
                in_=d_drow[:, C * CH:].to_broadcast((128, BPC)),
            )
            drow_sb = cp.tile([128, C * CH], bf16, name="drow", tag="drow")
            # first group's chars first so gather can start early
            nc.sync.dma_start(
                out=drow_sb[:, 0:gw[0] * CH],
                in_=d_drow[:, 0:gw[0] * CH].to_broadcast((128, gw[0] * CH)),
            )
            embp_sb = cp.tile([128, 2, E], bf16, name="embp", tag="embp")
            nc.sync.dma_start(
                out=embp_sb[:],
                in_=d_emb[:].rearrange("(c p) n -> p c n", p=128),
            )
            emb_sb = [embp_sb[:, c, :] for c in range(2)]

            pe_sb = []
            for g in range(G):
                t = cp.tile([128, 4 * gw[g] * CH], bf16, name=f"pe{g}",
                            tag=f"pe{g}")
                pe_sb.append(t)
            nc.sync.dma_start(
                out=pe_sb[0][:],
                in_=d_pe[:, 4 * CH * goff[0]:4 * CH * goff[1]],
            )

            wqp_sb = cp.tile([128, 4, D], bf16, name="wqp", tag="wqp")
            nc.sync.dma_start(
                out=wqp_sb[:],
                in_=d_wb[:, 0:D].rearrange("(c p) n -> p c n", p=128),
            )
            wqT_sb = [wqp_sb[:, e, :] for e in range(4)]

            id32_sb = cp.tile([32, 32], bf16, name="id32", tag="id32")
            nc.sync.dma_start(out=id32_sb[:], in_=d_id32[:])
            id4 = id32_sb[0:4, 0:4]

            if C > gw[0]:
                nc.sync.dma_start(
                    out=drow_sb[:, gw[0] * CH:],
                    in_=d_drow[:, gw[0] * CH:C * CH].to_broadcast(
                        (128, (C - gw[0]) * CH)),
                )
            if G > 1:
                nc.sync.dma_start(
                    out=pe_sb[1][:],
                    in_=d_pe[:, 4 * CH * goff[1]:4 * CH * goff[2]],
                )
            wkp_sb = cp.tile([128, 4, E], bf16, name="wkp", tag="wkp")
            nc.sync.dma_start(
                out=wkp_sb[:],
                in_=d_wb[:, D:2 * D].rearrange("(c p) n -> p c n", p=128),
            )
            wk_sb = [wkp_sb[:, c, :] for c in range(4)]

            mask_sb = cp.tile([128, C * NSC], bf16, name="mask", tag="mask")
            nc.sync.dma_start(out=mask_sb[:], in_=d_mask[:])

            wvp_sb = cp.tile([128, 4, D], bf16, name="wvp", tag="wvp")
            nc.sync.dma_start(
                out=wvp_sb[:],
                in_=d_wb[:, 2 * D:3 * D].rearrange("(c p) n -> p c n", p=128),
            )
            wvT_sb = [wvp_sb[:, e, :] for e in range(4)]

            for g in range(2, G):
                nc.sync.dma_start(
                    out=pe_sb[g][:],
                    in_=d_pe[:, 4 * CH * goff[g]:4 * CH * goff[g + 1]],
                )
            frow_sb = cp.tile([BPC, FR_W], f32, name="frow", tag="frow")
            nc.sync.dma_start(
                out=frow_sb[:], in_=d_frow[:].to_broadcast((BPC, FR_W))
            )
            b1row = frow_sb[:, 0:HS]
            b2row = frow_sb[:, HS:HS + NOUT]
            id1 = frow_sb[0:1, HS + NOUT:HS + NOUT + 1]

            wr_sb = cp.tile([128, 4, HS], f32r, name="wr", tag="wr")
            nc.sync.dma_start(
                out=wr_sb[:], in_=d_wr[:].rearrange("(c p) n -> p c n", p=128)
            )
            w1T_sb = [wr_sb[:, m, :] for m in range(4)]
            w2p_sb = cp.tile([128, 4, NOUT], bf16, name="w2p", tag="w2p")
            nc.sync.dma_start(
                out=w2p_sb[:],
                in_=d_w2[:].rearrange("(c p) n -> p c n", p=128),
            )
            w2T_sb = [w2p_sb[:, m, :] for m in range(4)]

            ones128 = cp.tile([128, 1], bf16, name="ones128", tag="ones128")
            nc.vector.memset(ones128[:], 1.0)

            # ---------------- gather helpers ----------------------------
            xT_sb = [[cp.tile([128, gw[g] * CH], bf16, name=f"xT{g}_{m}",
                              tag=f"xT{g}_{m}") for m in range(4)]
                     for g in range(G)]

            def emit_oh(g):
                oh = []
                for c in range(2):
                    o = wp.tile([128, gw[g] * CH], bf16, name=f"oh{g}_{c}",
                                tag="oh", bufs=6)
                    nc.vector.tensor_scalar(
                        o[:], drow_sb[:, goff[g] * CH:goff[g + 1] * CH],
                        cvals[:, c:c + 1], None, ALU.is_equal,
                    )
                    oh.append(o)
                return oh

            def emit_gather_block(g, m, oh):
                # gather e-block m of group g and evict (+pe) immediately
                p = psx.tile([128, gw[g] * CH], f32, name=f"xtp{g}_{m}",
                             tag="xtp")
                for c in range(2):
                    nc.tensor.matmul(
                        p[:], emb_sb[c][:, m * 128:(m + 1) * 128], oh[c][:],
                        start=(c == 0), stop=(c == 1),
                    )
                w = gw[g] * CH
                nc.vector.tensor_tensor(
                    xT_sb[g][m][:], p[:],
                    pe_sb[g][:, m * w:(m + 1) * w], ALU.add,
                )

            # ---------------- prologue ----------------------------------
            # x_last gather -> q (batched over the 4 sequences)
            ohl = []
            for c in range(2):
                t = cp.tile([128, BPC], bf16, name=f"ohl{c}", tag=f"ohl{c}")
                nc.vector.tensor_scalar(
                    t[:], idxl_sb[:], cvals[:, c:c + 1], None, ALU.is_equal
                )
                ohl.append(t)
            oh0 = emit_oh(0)
            xlast_sb = cp.tile([128, 4, BPC], bf16, name="xlast", tag="xlast")
            for m in range(4):
                p = pss.tile([128, BPC], f32, name=f"xlp{m}", tag="sp")
                for c in range(2):
                    nc.tensor.matmul(
                        p[:], emb_sb[c][:, m * 128:(m + 1) * 128], ohl[c][:],
                        start=(c == 0), stop=(c == 1),
                    )
                nc.vector.tensor_tensor(
                    xlast_sb[:, m, :], p[:], pelT[m], ALU.add
                )
            # gather group 0 early (needs only drow+emb+pe0 DMAs)
            for m in range(4):
                emit_gather_block(0, m, oh0)
            # q_all [4, 512] = x_last.T @ WqT   (bq is asserted zero)
            qp = psv.tile([BPC, D], f32, name="qp", tag="vp")
            for m in range(4):
                nc.tensor.matmul(
                    qp[:], xlast_sb[:, m, :], wqT_sb[m][:],
                    start=(m == 0), stop=(m == 3),
                )
            q_sb = cp.tile([BPC, D], f32, name="q_sb", tag="q_sb")
            nc.scalar.copy(q_sb[:], qp[:])
            # qT [128, 4(db), 4(s)]
            qT_sb = cp.tile([128, 4, BPC], f32, name="qT", tag="qT")
            for db in range(4):
                tp = pss.tile([128, BPC], f32, name=f"qTp{db}", tag="sp")
                nc.tensor.transpose(
                    tp[:], q_sb[:, db * 128:(db + 1) * 128], id4f
                )
                nc.vector.tensor_copy(qT_sb[:, db, :], tp[:])
            # group 1 gather
            if G > 1:
                oh1 = emit_oh(1)
                for m in range(4):
                    emit_gather_block(1, m, oh1)
            # qblk [128, 4(db), 32]: per (db, s) hmask * qT scalar column
            qblk_sb = cp.tile([128, 4, NSC], bf16, name="qblk", tag="qblk")
            for db in range(4):
                for s in range(BPC):
                    nc.vector.tensor_scalar(
                        qblk_sb[:, db, s * NH:(s + 1) * NH], hmask[db],
                        qT_sb[:, db, s:s + 1], None, ALU.mult,
                    )
            # qkv_all [32, 512e] = qblk.T @ Wk
            qkvp = psv.tile([NSC, E], f32, name="qkvp", tag="vp")
            for db in range(4):
                nc.tensor.matmul(
                    qkvp[:], qblk_sb[:, db, :], wk_sb[db][:],
                    start=(db == 0), stop=(db == 3),
                )
            qkv_sb = cp.tile([NSC, E], bf16, name="qkv_sb", tag="qkv_sb")
            nc.scalar.copy(qkv_sb[:], qkvp[:])
            # qkvT [128, 4(m), 32]
            qkvT_sb = cp.tile([128, 4, NSC], bf16, name="qkvT", tag="qkvT")
            for m in range(4):
                tp = pss.tile([128, NSC], bf16, name=f"qkvTp{m}", tag="sp")
                nc.tensor.transpose(
                    tp[:], qkv_sb[:, m * 128:(m + 1) * 128], id32_sb[:]
                )
                nc.vector.tensor_copy(qkvT_sb[:, m, :], tp[:])

            # ---------------- chunk loop --------------------------------
            ctxp = psc.tile([NSC, D], f32, name="ctxp", tag="cp")
            dnp = psd.tile([1, NSC], f32, name="dnp", tag="dn")

            pend_ctx = []  # (chunk_idx, aT, v) awaiting ctx/dn emission

            def emit_ctx_dn(force=False):
                while pend_ctx and (force or len(pend_ctx) > 1):
                    i, aT, v = pend_ctx.pop(0)
                    nc.tensor.matmul(
                        ctxp[:], aT[:], v[:],
                        start=(i == 0), stop=(i == C - 1),
                    )
                    nc.tensor.matmul(
                        dnp[:], ones128[:], aT[:],
                        start=(i == 0), stop=(i == C - 1),
                    )

            for g in range(G):
                # software-pipelined gather of group g+2, spread over cycle
                gl = g + 2
                if gl < G:
                    ohn = emit_oh(gl)
                    gq = [m for m in range(4)]
                else:
                    ohn, gq = None, []
                for j in range(gw[g]):
                    npop = (((j + 1) * 4 + gw[g] - 1) // gw[g]
                            - (j * 4 + gw[g] - 1) // gw[g]) if gq else 0
                    for _ in range(min(npop, len(gq))):
                        emit_gather_block(gl, gq.pop(0), ohn)
                    i = goff[g] + j
                    # scores + V share the xT stationary
                    slp = pss.tile([128, NSC], f32, name=f"slp{i}", tag="sp")
                    vp = psv.tile([128, D], f32, name=f"vp{i}", tag="vp")
                    for m in range(4):
                        stat = xT_sb[g][m][:, j * CH:(j + 1) * CH]
                        nc.tensor.matmul(
                            slp[:], stat, qkvT_sb[:, m, :],
                            start=(m == 0), stop=(m == 3),
                        )
                        nc.tensor.matmul(
                            vp[:], stat, wvT_sb[m][:],
                            start=(m == 0), stop=(m == 3),
                        )
                    slpm = wp.tile([128, NSC], f32, name=f"slpm{i}",
                                   tag="slpm", bufs=3)
                    nc.vector.tensor_tensor(
                        slpm[:], slp[:],
                        mask_sb[:, i * NSC:(i + 1) * NSC], ALU.add,
                    )
                    aT = wp.tile([128, NSC], bf16, name=f"aT{i}", tag="aT",
                                 bufs=4)
                    nc.scalar.activation(aT[:], slpm[:], AF.Exp, scale=SCALE)
                    v = wp.tile([128, D], bf16, name=f"v{i}", tag="v", bufs=3)
                    nc.scalar.copy(v[:], vp[:])
                    pend_ctx.append((i, aT, v))
                    emit_ctx_dn()
                while gq:
                    emit_gather_block(gl, gq.pop(0), ohn)
            emit_ctx_dn(force=True)

            # ---------------- softmax normalize + ctx.T ------------------
            dn_sb = wp.tile([1, NSC], f32, name="dn_sb", tag="dn_sb")
            nc.vector.tensor_copy(dn_sb[:], dnp[:])
            dTp = pss.tile([NSC, 1], f32, name="dTp", tag="sp")
            nc.tensor.transpose(dTp[:], dn_sb[:], id1)
            dsum = wp.tile([NSC, 1], f32, name="dsum", tag="dsum")
            nc.vector.tensor_copy(dsum[:], dTp[:])
            rec = wp.tile([NSC, 1], f32, name="rec", tag="rec")
            nc.vector.reciprocal(rec[:], dsum[:])
            ctx_sb = cp.tile([NSC, D], bf16, name="ctx_sb", tag="ctx_sb")
            nc.scalar.activation(ctx_sb[:], ctxp[:], AF.Copy, scale=rec[:])
            # ctxT4 [128, 4(db), 4(s)]: transpose blocks, head-select, reduce
            ctxT4 = cp.tile([128, 4, BPC], f32r, name="ctxT4", tag="ctxT4")
            for db in range(4):
                tp = pss.tile([128, NSC], bf16, name=f"ctp{db}", tag="sp")
                nc.tensor.transpose(
                    tp[:], ctx_sb[:, db * 128:(db + 1) * 128], id32_sb[:]
                )
                scr = wp.tile([128, BPC, NH], f32, name=f"scr{db}", tag="scr")
                nc.vector.tensor_tensor(scr[:], tp[:], hm32[db], ALU.mult)
                with nc.allow_low_precision("fp32 accum, fp32r round"):
                    nc.vector.tensor_reduce(
                        ctxT4[:, db, :], scr[:], mybir.AxisListType.X, ALU.add
                    )

            # ---------------- prediction head (batched) ------------------
            hp = psv.tile([BPC, HS], f32, name="hp", tag="vp")
            for db in range(4):
                nc.tensor.matmul(
                    hp[:], ctxT4[:, db, :], w1T_sb[db][:],
                    start=(db == 0), stop=(db == 3),
                )
            h1 = wp.tile([BPC, HS], f32, name="h1", tag="h1")
            nc.vector.tensor_tensor(h1[:], hp[:], b1row, ALU.add)
            h_sb = cp.tile([BPC, HS], bf16, name="h_sb", tag="h_sb")
            nc.vector.scalar_tensor_tensor(
                h_sb[:], h1[:], 0.01, h1[:], ALU.mult, ALU.max
            )
            hT_sb = cp.tile([128, 4, BPC], bf16, name="hT", tag="hT")
            for m in range(4):
                tp = pss.tile([128, BPC], bf16, name=f"hTp{m}", tag="sp")
                nc.tensor.transpose(
                    tp[:], h_sb[:, m * 128:(m + 1) * 128], id4
                )
                nc.vector.tensor_copy(hT_sb[:, m, :], tp[:])
            r2p = pss.tile([BPC, NOUT], f32, name="r2p", tag="sp")
            for m in range(4):
                nc.tensor.matmul(
                    r2p[:], hT_sb[:, m, :], w2T_sb[m][:],
                    start=(m == 0), stop=(m == 3),
                )
            r2 = wp.tile([BPC, NOUT], f32, name="r2", tag="r2")
            nc.vector.tensor_tensor(r2[:], r2p[:], b2row, ALU.add)
            r2r = wp.tile([BPC, NOUT], f32, name="r2r", tag="r2r")
            nc.vector.tensor_scalar(r2r[:], r2[:], 0.0, None, ALU.max)
            mt = wp.tile([BPC, 1], f32, name="mt", tag="mt")
            nc.vector.tensor_reduce(
                mt[:], r2r[:], mybir.AxisListType.X, ALU.add
            )
            ms = wp.tile([BPC, 1], f32, name="ms", tag="ms")
            nc.vector.tensor_scalar(ms[:], mt[:], 1.0 / NOUT, None, ALU.mult)
            out_sb = cp.tile([BPC, 1], f32, name="out_sb", tag="out_sb")
            nc.vector.scalar_tensor_tensor(
                out_sb[:], ms[:], 0.01, ms[:], ALU.mult, ALU.max
            )
            nc.sync.dma_start(out=d_out[:], in_=out_sb[:])

    nc.compile()
    return nc


_CACHE = {}


def _get_module(C):
    if C not in _CACHE:
        _CACHE[C] = _build(C)
    return _CACHE[C]


def _pos_encoding(n):
    pos = np.arange(n, dtype=np.float32)[:, None]
    div = np.exp(
        np.arange(0, D, 2, dtype=np.float32) * (-math.log(10000.0) / D)
    )
    pe = np.zeros((n, D), np.float32)
    pe[:, 0::2] = np.sin(pos * div)
    pe[:, 1::2] = np.cos(pos * div)
    return pe


def _plan(lengths):
    """LPT-balance sequences into 8 groups of 4 by chunk count."""
    nch = -(-lengths // CH)  # ceil
    order = np.argsort(-nch, kind="stable")
    groups = [[] for _ in range(N_CORES)]
    sums = [0] * N_CORES
    for idx in order:
        cands = [g for g in range(N_CORES) if len(groups[g]) < BPC]
        g = min(cands, key=lambda g: sums[g])
        groups[g].append(int(idx))
        sums[g] += int(nch[idx])
    return groups, int(max(sums)), nch


def make_in_maps(data, lengths, emb, Wq, bq, Wk, bk, Wv, bv, W1, b1, W2, b2):
    # the kernel folds the K projection into the score contraction and
    # skips the q/v biases entirely; all three are zero for this module.
    assert float(np.abs(np.asarray(bq)).max()) == 0.0
    assert float(np.abs(np.asarray(bk)).max()) == 0.0
    assert float(np.abs(np.asarray(bv)).max()) == 0.0

    data = np.asarray(data)
    lengths = np.asarray(lengths).astype(np.int64)
    groups, C, nch = _plan(lengths)
    gwl = _group_widths(C)
    p = lengths - 1

    bfl = ml_dtypes.bfloat16
    pe = _pos_encoding(1024)                       # [1024, D]

    wb = np.concatenate(
        [np.asarray(Wq).T, np.asarray(Wk), np.asarray(Wv).T], axis=1
    ).astype(bfl)                                  # [512, 1536]
    wr = np.ascontiguousarray(np.asarray(W1).T, dtype=np.float32)  # [D, HS]
    w2t = np.ascontiguousarray(np.asarray(W2).T, dtype=bfl)        # [HS, 8]

    # fmix: cvals | hmask | pelT | hm32 (pelT filled per-core below)
    fmix0 = np.zeros((128, FM_W), np.float32)
    fmix0[:, FM_CV + 0] = np.arange(128)
    fmix0[:, FM_CV + 1] = np.arange(128, 256)
    rows = np.arange(128)
    for db in range(4):
        hm = np.zeros((128, NH), np.float32)
        hm[rows, 2 * db + rows // 64] = 1.0
        fmix0[:, FM_HM + 8 * db:FM_HM + 8 * (db + 1)] = hm
        fmix0[:, FM_H32 + NSC * db:FM_H32 + NSC * (db + 1)] = np.tile(
            hm, (1, BPC)
        )
    fmix0[0:4, FM_ID4:FM_ID4 + 4] = np.eye(4)

    frow = np.zeros((1, FR_W), np.float32)
    frow[0, 0:HS] = np.asarray(b1)
    frow[0, HS:HS + NOUT] = np.asarray(b2)
    frow[0, HS + NOUT] = 1.0

    dpad = np.full((B, 1024), SENT, np.float32)
    dpad[:, :L] = data[:, :L]

    shared = {
        "emb": np.ascontiguousarray(emb, dtype=bfl),
        "wb": np.ascontiguousarray(wb),
        "wr": wr,
        "w2": w2t,
        "id32": np.eye(32, dtype=bfl),
        "frow": frow,
    }
    in_maps = []
    for c in range(N_CORES):
        seqs = groups[c]
        # chunk list: (owner_slot, k) in sequence-major order + pads
        chunks = []
        for s, b in enumerate(seqs):
            chunks += [(s, k) for k in range(int(nch[b]))]
        chunks += [(-1, 0)] * (C - len(chunks))

        drow = np.full(C * CH + BPC, SENT, np.float32)
        mask = np.full((128, C, NSC), NEG, np.float32)
        # pe packed per group as [128, 4(m), gw(j), CH]
        pe_parts = []
        gi0 = 0
        for w in gwl:
            arr = np.zeros((128, 4, w, CH), np.float32)
            for j in range(w):
                i = gi0 + j
                s, k = chunks[i]
                if s >= 0:
                    pe_blk = pe[k * CH:(k + 1) * CH, :]   # [128 l, 512 e]
                    arr[:, :, j, :] = pe_blk.T.reshape(
                        4, 128, CH).transpose(1, 0, 2)
            pe_parts.append(arr.reshape(128, 4 * w * CH))
            gi0 += w
        for i, (s, k) in enumerate(chunks):
            if s < 0:
                continue
            b = seqs[s]
            drow[i * CH:(i + 1) * CH] = dpad[b, k * CH:(k + 1) * CH]
            lpos = k * CH + np.arange(CH)
            valid = lpos <= p[b]                          # [128]
            mask[:, i, s * NH:(s + 1) * NH] = np.where(
                valid[:, None], 0.0, NEG
            )
        drow[C * CH:] = data[np.arange(B), p][seqs]

        fmix = fmix0.copy()
        pl = pe[p[seqs], :]                               # [4, 512]
        for m in range(4):
            fmix[:, FM_PL + 4 * m:FM_PL + 4 * (m + 1)] = (
                pl[:, m * 128:(m + 1) * 128].T
            )

        m = dict(shared)
        m["drow"] = drow.reshape(1, -1).astype(bfl)
        m["pe"] = np.ascontiguousarray(
            np.concatenate(pe_parts, axis=1)).astype(bfl)
        m["mask"] = np.ascontiguousarray(
            mask.reshape(128, C * NSC)).astype(bfl)
        m["fmix"] = np.ascontiguousarray(fmix)
        in_maps.append(m)
    return in_maps, groups, C


def kernel(data, lengths, emb, Wq, bq, Wk, bk, Wv, bv, W1, b1, W2, b2):
    in_maps, groups, C = make_in_maps(
        data, lengths, emb, Wq, bq, Wk, bk, Wv, bv, W1, b1, W2, b2
    )
    nc = _get_module(C)
    res = run_bass_kernel_spmd(nc, in_maps, list(range(N_CORES)))
    out = np.zeros(B, np.float32)
    for c in range(N_CORES):
        vals = res.results[c]["out"].reshape(BPC)
        for s, b in enumerate(groups[c]):
            out[b] = vals[s]
    return out
